# revision 1
# baseline (speedup 1.0000x reference)
"""ColorNorm Trainium2 kernel.

Problem: per-sample 3x3 color-matching solve over N=1024*1024 pixels.
  A = src[b] (3,N), B = dst[b] (3,N)
  AAt = Ac@Ac.T + 1e-3 I ; BAt = Bc@Ac.T ; x = BAt@inv(AAt)
  out[b] = x@Ac + Bmean
Sharding: data-parallel over batch (16 samples -> 8 cores x 2 samples).

Per-core pipeline (fp16 data plane, fp32 accumulation/solve/output):
  load:  A,B cast fp32->fp16 in-flight by SWDGE DMA; A stays resident.
  pass1: DVE tensor_mul (fp16 2x) computes the 12 cross products; their
         reduction runs on PE as ones-matmuls accumulating into col-tiled
         PSUM [1,512] partials (finished by a tiny partition_all_reduce).
         ScalarE Square+accum does the 3 diagonals; DVE tensor_scalar+accum
         (4x fp16) does raw channel sums; one ones-matmul reduces those
         across partitions.
  solve: 3x3 inverse via adjugate (tiny fp32 DVE ops on partition 0).
  pass2: out_i = sum_j x_ij*A_j + d_i via fp16 PE matmuls with diag(x_ij)
         stationary weights accumulating in fp32 PSUM; ScalarE evicts with
         the +d_i bias fused; 1MiB fp32 store DMAs.
"""

import sys

for _p in ("/opt/trn_rl_repo", "/opt/pypackages"):
    if _p not in sys.path:
        sys.path.append(_p)

from contextlib import ExitStack

import numpy as np

import concourse.bacc as bacc
import concourse.bass as bass
import concourse.tile as tile
from concourse import bass_isa, masks, mybir
from concourse._compat import with_exitstack

# ---- hardcoded problem geometry (per core) ----
B_CORE = 2          # samples per core
C = 3               # channels
H = W = 1024
N = H * W           # 1048576 pixels per channel
P = 128             # SBUF partitions
F = N // P          # 8192 free elems per partition per channel
Q = 2048            # quarter-chunk free size
NQ = F // Q         # 4 quarters
HB = 4096           # B half-channel free size
MM = 512            # matmul free-dim chunk (one PSUM bank)
NCORES = 8
RIDGE = 1e-3
import os
GP_N = int(os.environ.get("CN_GP_N", "0"))
B_BUFS = int(os.environ.get("CN_B_BUFS", "4"))
PS_BUFS = int(os.environ.get("CN_PS_BUFS", "3"))
PE_RED = int(os.environ.get("CN_PE_RED", "1"))
PTW = int(os.environ.get("CN_PTW", "512"))
SCRB_BUFS = int(os.environ.get("CN_SCRB", "4"))
A_BUFS = int(os.environ.get("CN_A_BUFS", "6"))

F32 = mybir.dt.float32
F16 = mybir.dt.float16
ALU = mybir.AluOpType
ACTF = mybir.ActivationFunctionType

# 6 unique AA pairs; symmetric index map
A_PAIRS = [(0, 0), (0, 1), (0, 2), (1, 1), (1, 2), (2, 2)]
SYM = {(0, 0): 0, (0, 1): 1, (0, 2): 2, (1, 1): 3, (1, 2): 4, (2, 2): 5}


def _rd(ap, dims):
    """Rebuild an AP keeping its partition dim, replacing free dims."""
    return bass.AP(ap.tensor, ap.offset, [ap.ap[0]] + dims)


@with_exitstack
def _colornorm(ctx: ExitStack, tc: "tile.TileContext", src, dst, out):
    nc = tc.nc
    srcv = src.rearrange("b c (p q) w -> b c p (q w)", p=P)  # [2,3,128,8192]
    dstv = dst.rearrange("b c (p q) w -> b c p (q w)", p=P)
    outv = out.rearrange("b c (p q) w -> b c p (q w)", p=P)

    singles = ctx.enter_context(tc.tile_pool(name="singles", bufs=1))
    a_pool = ctx.enter_context(tc.tile_pool(name="a_pool", bufs=A_BUFS))
    b_pool = ctx.enter_context(tc.tile_pool(name="b_pool", bufs=B_BUFS))
    scr_pool = ctx.enter_context(tc.tile_pool(name="scr", bufs=1))
    scrb_pool = ctx.enter_context(tc.tile_pool(name="scrb", bufs=SCRB_BUFS))
    gscr_pool = ctx.enter_context(tc.tile_pool(name="gscr", bufs=1))
    ascr_pool = ctx.enter_context(tc.tile_pool(name="ascr", bufs=1))
    acc_pool = ctx.enter_context(tc.tile_pool(name="accs", bufs=2))
    solve_pool = ctx.enter_context(tc.tile_pool(name="solve", bufs=2))
    dg_pool = ctx.enter_context(tc.tile_pool(name="dg", bufs=1))
    stage_pool = ctx.enter_context(tc.tile_pool(name="stage", bufs=2))
    ps_stat = ctx.enter_context(tc.tile_pool(name="ps_stat", bufs=2, space="PSUM"))
    ps_acc = ctx.enter_context(tc.tile_pool(name="ps_acc", bufs=3, space="PSUM"))
    ps_out = ctx.enter_context(tc.tile_pool(name="ps_out", bufs=PS_BUFS, space="PSUM"))

    ones = singles.tile([P, 1], F32)
    nc.vector.memset(ones, 1.0)
    ones16 = singles.tile([P, 1], F16)
    nc.vector.memset(ones16, 1.0)
    eye = singles.tile([P, P], F16)
    masks.make_identity(nc, eye[:])

    for s in range(B_CORE):
        # ------------- load (fp32 -> fp16 cast in DMA) -------------
        a_t = [a_pool.tile([P, F], F16, tag="ach", name="ach") for _ in range(C)]
        for c in range(C):
            nc.gpsimd.dma_start(out=a_t[c][:], in_=srcv[s, c])
        b_t = [[None, None] for _ in range(C)]
        for c in range(C):
            for h in range(2):
                b_t[c][h] = b_pool.tile([P, HB], F16, tag="bh", name="bh")
                nc.gpsimd.dma_start(out=b_t[c][h][:],
                                    in_=dstv[s, c][:, h * HB:(h + 1) * HB])

        def aq(c, q):  # quarter view of A channel
            return a_t[c][:, q * Q:(q + 1) * Q]

        def bh(c, h):  # half view of B
            return b_t[c][h][:]

        # merged per-partition accumulator columns:
        #  0-2   A cross pairs (0,1),(0,2),(1,2)   [full channel]
        #  3-5   A diag (ScalarE Square)           [full channel]
        #  6-23  BA pairs (c,j) x half h: 6+2*(3c+j)+h
        #  24-26 raw A sums [full channel]
        #  27-32 raw B sums (c,h): 27+2c+h
        acc = acc_pool.tile([P, 33], F32, tag="acc", name="acc")
        A_CROSS = [(0, 1), (0, 2), (1, 2)]
        # BA pairs whose multiply runs on GpSimd (idle otherwise)
        GP_BA = set([1, 3, 5, 7, 0, 8][:GP_N])

        def prod(col, x_ap, y_ap, width, pool_mul=False):
            # fp16 TT runs 2x, single-src ts+accum runs 4x; the fused
            # 2-input STT would be 1x — two fast ops beat one slow one.
            # pool_mul pushes the multiply to GpSimd (library TensorTensor).
            if pool_mul:
                scr = gscr_pool.tile([P, HB], F16, tag="gscr", name="gscr")
                nc.gpsimd.tensor_mul(out=scr[:, 0:width], in0=x_ap, in1=y_ap)
            else:
                scr = scr_pool.tile([P, F], F16, tag="scr", name="scr")
                nc.vector.tensor_mul(out=scr[:, 0:width], in0=x_ap, in1=y_ap)
            nc.vector.tensor_scalar(
                out=scr[:, 0:width], in0=scr[:, 0:width], scalar1=1.0,
                scalar2=0.0, op0=ALU.mult, op1=ALU.add,
                accum_out=acc[:, col:col + 1])

        # PE-reduce variant state: pair k -> psum bank k//4, col-group k%4
        psa = [ps_acc.tile([P, MM], F32, tag="psa", name="psa")
               for _ in range(3)] if PE_RED else None
        pair_mm_seen = [0] * 12
        PAIR_MMS = [16] * 3 + [16] * 9  # total 512-col mms per pair

        def prod_pe(k, x_ap, y_ap, width):
            """multiply on DVE; reduce via ones-matmul into psum[1,512]
            at partition 32*(k%4) of bank k//4 (PSUM accumulation).
            Separate scratch pool: PE is the consumer, so these tiles
            must not throttle DVE's next multiply."""
            scr = scrb_pool.tile([P, HB], F16, tag="scrb", name="scrb")
            nc.vector.tensor_mul(out=scr[:, 0:width], in0=x_ap, in1=y_ap)
            bank, grp = k // 4, k % 4
            for m in range(width // MM):
                first = pair_mm_seen[k] == 0
                pair_mm_seen[k] += 1
                last = pair_mm_seen[k] == PAIR_MMS[k]
                nc.tensor.matmul(
                    psa[bank][32 * grp:32 * grp + 1, :], ones16[:],
                    scr[:, m * MM:(m + 1) * MM],
                    start=first, stop=last,
                    tile_position=(0, 32 * grp))

        def raw_sum(col, x_ap, width):
            scr = scr_pool.tile([P, F], F16, tag="scr", name="scr")
            nc.vector.tensor_scalar(
                out=scr[:, 0:width], in0=x_ap, scalar1=1.0, scalar2=0.0,
                op0=ALU.mult, op1=ALU.add,
                accum_out=acc[:, col:col + 1])

        # A-channel stats (full-channel ops)
        asq = acc_pool.tile([P, 6], F32, tag="asq", name="asq")
        for c in range(C):
            raw_sum(24 + c, a_t[c][:], F)
            for h in range(2):
                ascr = ascr_pool.tile([P, HB], F16, tag="ascr", name="ascr")
                nc.scalar.activation(
                    out=ascr[:], in_=a_t[c][:, h * HB:(h + 1) * HB],
                    func=ACTF.Square, accum_out=asq[:, 2 * c + h: 2 * c + h + 1])
        # fold the square halves into acc cols 3..5 (tiny)
        nc.vector.reduce_sum(out=acc[:, 3:6].rearrange("p (c o) -> p c o", o=1),
                             in_=asq[:, 0:6].rearrange("p (c h) -> p c h", h=2),
                             axis=mybir.AxisListType.X)
        for k, (i, j) in enumerate(A_CROSS):
            prod(k, a_t[i][:], a_t[j][:], F)
        # B stats (half-channel granularity to pipeline with B loads)
        for c in range(C):
            for h in range(2):
                raw_sum(27 + 2 * c + h, bh(c, h), HB)
                for j in range(C):
                    if PE_RED:
                        prod_pe(3 + 3 * c + j, bh(c, h),
                                a_t[j][:, h * HB:(h + 1) * HB], HB)
                    else:
                        prod(6 + 2 * (3 * c + j) + h,
                             bh(c, h), a_t[j][:, h * HB:(h + 1) * HB], HB,
                             pool_mul=(3 * c + j) in GP_BA)

        # cross-partition reduce on PE: ones.T @ acc -> [1, 33]
        pst = ps_stat.tile([1, 40], F32, tag="pst", name="pst")
        nc.tensor.matmul(pst[0:1, 0:33], ones[:], acc[:],
                         start=True, stop=True)
        stats = solve_pool.tile([1, 40], F32, tag="stats", name="stats")
        nc.vector.tensor_copy(out=stats[0:1, 0:33], in_=pst[0:1, 0:33])
        BA9 = solve_pool.tile([1, 9], F32, tag="BA9", name="BA9")
        if PE_RED:
            # pair partials live at [1,512] psum regions; reduce each to a
            # scalar at its partition, then all-reduce across partitions
            prow = solve_pool.tile([P, 12], F32, tag="prow", name="prow")
            nc.vector.memset(prow[:], 0.0)
            for k in range(12):
                bank, grp = k // 4, k % 4
                nc.vector.reduce_sum(
                    out=prow[32 * grp:32 * grp + 1, k:k + 1],
                    in_=psa[bank][32 * grp:32 * grp + 1, :],
                    axis=mybir.AxisListType.X)
            prow2 = solve_pool.tile([P, 12], F32, tag="prow2", name="prow2")
            nc.gpsimd.partition_all_reduce(
                prow2[:], prow[:], channels=P,
                reduce_op=bass_isa.ReduceOp.add)
            nc.vector.tensor_copy(out=BA9[:], in_=prow2[0:1, 3:12])
        else:
            # BA pair sums: collapse the two half partials
            nc.vector.reduce_sum(out=BA9[:], axis=mybir.AxisListType.X,
                                 in_=stats[0:1, 6:24].rearrange(
                                     "p (k h) -> p k h", h=2))

        # ---------------- 3x3 solve on partition 0 ----------------
        sumB = solve_pool.tile([1, 3], F32, tag="sumB", name="sumB")
        nc.vector.reduce_sum(out=sumB[:], axis=mybir.AxisListType.X,
                             in_=stats[0:1, 27:33].rearrange(
                                 "p (c h) -> p c h", h=2))
        Am = solve_pool.tile([1, 3], F32, tag="Am", name="Am")
        Bm = solve_pool.tile([1, 3], F32, tag="Bm", name="Bm")
        nc.vector.tensor_scalar_mul(out=Am[:], in0=stats[0:1, 24:27],
                                    scalar1=1.0 / N)
        nc.vector.tensor_scalar_mul(out=Bm[:], in0=sumB[:], scalar1=1.0 / N)

        AA9 = solve_pool.tile([1, 9], F32, tag="AA9", name="AA9")
        SYM3 = {(0, 1): 0, (0, 2): 1, (1, 2): 2}
        for i in range(C):
            for j in range(C):
                col = 3 + i if i == j else SYM3[(min(i, j), max(i, j))]
                nc.vector.tensor_copy(out=AA9[0:1, 3 * i + j: 3 * i + j + 1],
                                      in_=stats[0:1, col:col + 1])

        # centered: AAc = AA - N*Am Am^T (+ridge); BAc = BA - N*Bm Am^T
        outer = solve_pool.tile([1, 9], F32, tag="outer", name="outer")
        o3x3 = outer[0:1, :].rearrange("p (i j) -> p i j", j=3)
        nc.vector.tensor_mul(out=o3x3, in0=_rd(Am[0:1, 0:1], [[1, 3], [0, 3]]),
                             in1=_rd(Am[0:1, 0:1], [[0, 3], [1, 3]]))
        AAc = solve_pool.tile([1, 9], F32, tag="AAc", name="AAc")
        nc.vector.scalar_tensor_tensor(out=AAc[:], in0=outer[:],
                                       scalar=-float(N), in1=AA9[:],
                                       op0=ALU.mult, op1=ALU.add)
        dg_ap = _rd(AAc[0:1, 0:1], [[4, 3]])
        nc.vector.tensor_scalar_add(out=dg_ap, in0=dg_ap, scalar1=RIDGE)
        nc.vector.tensor_mul(out=o3x3, in0=_rd(Bm[0:1, 0:1], [[1, 3], [0, 3]]),
                             in1=_rd(Am[0:1, 0:1], [[0, 3], [1, 3]]))
        BAc = solve_pool.tile([1, 9], F32, tag="BAc", name="BAc")
        nc.vector.scalar_tensor_tensor(out=BAc[:], in0=outer[:],
                                       scalar=-float(N), in1=BA9[:],
                                       op0=ALU.mult, op1=ALU.add)

        # inverse via adjugate: M2 = 6x6 tiling of AAc (mod-3 access)
        M2 = solve_pool.tile([1, 36], F32, tag="M2", name="M2")
        for dr in (0, 3):
            for dc in (0, 3):
                nc.vector.tensor_copy(
                    out=_rd(M2[0:1, 6 * dr + dc: 6 * dr + dc + 1],
                            [[6, 3], [1, 3]]),
                    in_=AAc[0:1, :].rearrange("p (i j) -> p i j", j=3))
        t1 = solve_pool.tile([1, 9], F32, tag="t1", name="t1")
        t2 = solve_pool.tile([1, 9], F32, tag="t2", name="t2")
        nc.vector.tensor_mul(out=t1[0:1, :].rearrange("p (i j) -> p i j", j=3),
                             in0=_rd(M2[0:1, 7:8], [[6, 3], [1, 3]]),
                             in1=_rd(M2[0:1, 14:15], [[6, 3], [1, 3]]))
        nc.vector.tensor_mul(out=t2[0:1, :].rearrange("p (i j) -> p i j", j=3),
                             in0=_rd(M2[0:1, 8:9], [[6, 3], [1, 3]]),
                             in1=_rd(M2[0:1, 13:14], [[6, 3], [1, 3]]))
        cof = solve_pool.tile([1, 9], F32, tag="cof", name="cof")
        nc.vector.tensor_sub(out=cof[:], in0=t1[:], in1=t2[:])

        det = solve_pool.tile([1, 1], F32, tag="det", name="det")
        dscr = solve_pool.tile([1, 3], F32, tag="dscr", name="dscr")
        nc.vector.scalar_tensor_tensor(
            out=dscr[:], in0=AAc[0:1, 0:3], scalar=1.0, in1=cof[0:1, 0:3],
            op0=ALU.mult, op1=ALU.mult, accum_out=det[:])
        rdet = solve_pool.tile([1, 1], F32, tag="rdet", name="rdet")
        nc.vector.reciprocal(out=rdet[:], in_=det[:])

        inv9 = solve_pool.tile([1, 9], F32, tag="inv9", name="inv9")
        nc.vector.tensor_scalar_mul(
            out=inv9[0:1, :].rearrange("p (i j) -> p i j", j=3),
            in0=_rd(cof[0:1, 0:1], [[1, 3], [3, 3]]),  # cof^T
            scalar1=rdet[:])

        # x = BAc @ inv  (tmp27[i,k,j] = BAc[i,j]*inv[j,k], reduce j)
        tmp27 = solve_pool.tile([1, 27], F32, tag="tmp27", name="tmp27")
        nc.vector.tensor_mul(
            out=tmp27[0:1, :].rearrange("p (i k j) -> p i k j", k=3, j=3),
            in0=_rd(BAc[0:1, 0:1], [[3, 3], [0, 3], [1, 3]]),
            in1=_rd(inv9[0:1, 0:1], [[0, 3], [1, 3], [3, 3]]))
        x9 = solve_pool.tile([1, 9], F32, tag="x9", name="x9")
        nc.vector.reduce_sum(
            out=x9[0:1, :].rearrange("p (i k) -> p i k", k=3),
            in_=tmp27[0:1, :].rearrange("p (i k j) -> p i k j", k=3, j=3),
            axis=mybir.AxisListType.X)

        # d = Bm - x@Am
        tmp9 = solve_pool.tile([1, 9], F32, tag="tmp9", name="tmp9")
        nc.vector.tensor_mul(
            out=tmp9[0:1, :].rearrange("p (i j) -> p i j", j=3),
            in0=x9[0:1, :].rearrange("p (i j) -> p i j", j=3),
            in1=_rd(Am[0:1, 0:1], [[0, 3], [1, 3]]))
        xAm = solve_pool.tile([1, 3], F32, tag="xAm", name="xAm")
        nc.vector.reduce_sum(out=xAm[:], axis=mybir.AxisListType.X,
                             in_=tmp9[0:1, :].rearrange("p (i j) -> p i j", j=3))
        sol = solve_pool.tile([1, 12], F32, tag="sol", name="sol")
        nc.vector.tensor_copy(out=sol[0:1, 0:9], in_=x9[:])
        nc.vector.tensor_sub(out=sol[0:1, 9:12], in0=Bm[:], in1=xAm[:])

        # broadcast x,d to all partitions
        xb = solve_pool.tile([P, 12], F32, tag="xb", name="xb")
        nc.gpsimd.partition_broadcast(xb[:], sol[0:1, 0:12])

        # diag(x_ij) fp16 weight tiles
        dg = [[dg_pool.tile([P, P], F16, tag=f"dg{i}{j}", name=f"dg{i}{j}")
               for j in range(C)] for i in range(C)]
        for i in range(C):
            for j in range(C):
                nc.vector.tensor_scalar_mul(
                    out=dg[i][j][:], in0=eye[:],
                    scalar1=xb[:, 3 * i + j: 3 * i + j + 1])

        # -------- pass 2: out_i = sum_j x_ij A_j + d_i --------
        # For the last sample the tail has no other work: DVE computes
        # channel 0 (ts+stt chain) while PE does channels 1,2.
        dve_ch = {0} if (PE_RED and s == B_CORE - 1) else set()
        for g in range(NQ):
            for i in range(C):
                if i in dve_ch:
                    stage = stage_pool.tile([P, Q], F32, tag="stage",
                                            name="stage")
                    ga = slice(g * Q, (g + 1) * Q)
                    nc.vector.tensor_scalar(
                        out=stage[:], in0=a_t[0][:, ga],
                        scalar1=xb[:, 3 * i: 3 * i + 1],
                        scalar2=xb[:, 9 + i: 10 + i],
                        op0=ALU.mult, op1=ALU.add)
                    for j in (1, 2):
                        nc.vector.scalar_tensor_tensor(
                            out=stage[:], in0=a_t[j][:, ga],
                            scalar=xb[:, 3 * i + j: 3 * i + j + 1],
                            in1=stage[:], op0=ALU.mult, op1=ALU.add)
                    nc.scalar.dma_start(out=outv[s, i][:, ga], in_=stage[:])
                    continue
                stage = stage_pool.tile([P, Q], F32, tag="stage", name="stage")
                for hh in range(Q // PTW):
                    pt = ps_out.tile([P, PTW], F32, tag="pt", name="pt")
                    for j in range(C):
                        for cc in range(PTW // MM):
                            o0 = hh * PTW + cc * MM
                            nc.tensor.matmul(
                                pt[:, cc * MM:(cc + 1) * MM], dg[i][j][:],
                                a_t[j][:, g * Q + o0: g * Q + o0 + MM],
                                start=(j == 0), stop=(j == 2))
                    nc.scalar.add(out=stage[:, hh * PTW:(hh + 1) * PTW],
                                  in_=pt[:], add=xb[:, 9 + i: 10 + i])
                nc.scalar.dma_start(out=outv[s, i][:, g * Q:(g + 1) * Q],
                                    in_=stage[:])


def build_nc() -> "bass.Bass":
    nc = bacc.Bacc("TRN2", target_bir_lowering=False)
    src = nc.dram_tensor("src", [B_CORE, C, H, W], F32, kind="ExternalInput")
    dst = nc.dram_tensor("dst", [B_CORE, C, H, W], F32, kind="ExternalInput")
    out = nc.dram_tensor("out", [B_CORE, C, H, W], F32, kind="ExternalOutput")
    with tile.TileContext(nc) as tc:
        _colornorm(tc, src[:], dst[:], out[:])
    nc.finalize()
    return nc


_NC = None


def _get_nc():
    global _NC
    if _NC is None:
        _NC = build_nc()
    return _NC


TRACE = False
LAST_RESULT = None  # BassKernelResults of the most recent run (for profiling)


def kernel(src, dst):
    from concourse.bass_utils import run_bass_kernel_spmd

    global LAST_RESULT
    src = np.ascontiguousarray(np.asarray(src, dtype=np.float32))
    dst = np.ascontiguousarray(np.asarray(dst, dtype=np.float32))
    assert src.shape == (NCORES * B_CORE, C, H, W), src.shape
    nc = _get_nc()
    in_maps = [
        {
            "src": np.ascontiguousarray(src[i * B_CORE:(i + 1) * B_CORE]),
            "dst": np.ascontiguousarray(dst[i * B_CORE:(i + 1) * B_CORE]),
        }
        for i in range(NCORES)
    ]
    res = run_bass_kernel_spmd(nc, in_maps, core_ids=list(range(NCORES)),
                               trace=TRACE)
    LAST_RESULT = res
    return np.concatenate([r["out"] for r in res.results], axis=0)

